# revision 10
# baseline (speedup 1.0000x reference)
"""Trainium2 Bass kernel for AttnPainterOilDensity (per-pixel top-10 stroke
selection + back-to-front alpha compositing).

Math (per pixel, strokes n = 0..255):
  m_n   = alpha_n > 0.1
  E_n   = #{k > n : m_k}                      (visible strokes in front)
  sel_n = m_n and E_n <= 9                    (the last 10 visible strokes)
  ae_n  = alpha_n * sel_n
  lg_n  = ln(1 - ae_n)                        (0 for unselected)
  Lx_n  = sum_{k>n} lg_k ;  Texcl_n = exp(Lx_n)
  w_n   = ae_n * Texcl_n                      (compositing weight)
  canvas_c = sum_n w_n * color_{n,c} + exp(sum_n lg_n)
  den      = sum_n w_n * s_n        + exp(sum_n lg_n),  s_n = p2_n * p3_n

Layout: stroke-major tiles [128 strokes, 512 pixels]; the per-pixel suffix
counts/sums over the stroke (partition) axis run on the PE via triangular
constant stationaries; selection via one fused scalar_tensor_tensor; ln/exp
on ACT; per-stroke reductions back to per-pixel rows via small matmuls.

Sharding: 8 cores = (batch b = core//2) x (half of the 128x128 plane).
"""

import numpy as np
import ml_dtypes

import concourse.bacc as bacc
import concourse.bass as bass
import concourse.tile as tile
from concourse import mybir
from concourse.bass_utils import run_bass_kernel_spmd


def _patch_act_tables():
    # Force Ln and Exp onto the shared natural_log_exp_and_others set so the
    # per-tile Ln -> Exp alternation doesn't reload ACT tables 2x per tile.
    if _CACHED.get("act_patched"):
        return
    import concourse.hw_specs as hw_specs
    orig = hw_specs.get_activation_tables

    def patched(arch):
        tables = dict(orig(arch))
        ln = mybir.ActivationFunctionType.Ln
        ex = mybir.ActivationFunctionType.Exp
        for name, fns in tables.items():
            if name != "natural_log_exp_and_others":
                tables[name] = fns - {ln, ex}
        return tables

    hw_specs.get_activation_tables = patched
    bacc.get_activation_tables = patched
    _CACHED["act_patched"] = True

B, N, H, W = 4, 256, 128, 128
PIX = H * W // 2          # pixels per core (half plane) = 8192
F = 512                   # pixels per tile
NT = PIX // F             # 16 tiles
BIG = 1024.0
THRESH = -1014.5          # q <= 9 - BIG

f32 = mybir.dt.float32
f32r = mybir.dt.float32r
bf16 = mybir.dt.bfloat16

_CACHED = {}


def _build_program():
    _patch_act_tables()
    nc = bacc.Bacc("TRN2", target_bir_lowering=False, debug=False, num_devices=8)

    a_d = nc.dram_tensor("alpha", [N, PIX], f32, kind="ExternalInput")
    c_d = nc.dram_tensor("color", [N, 3, PIX], f32, kind="ExternalInput")
    p_d = nc.dram_tensor("par", [N, 8], f32, kind="ExternalInput")
    u2b_d = nc.dram_tensor("U2B", [128, 128], bf16, kind="ExternalInput")
    onb_d = nc.dram_tensor("ONESB", [128, 128], bf16, kind="ExternalInput")
    thrh_d = nc.dram_tensor("THRH", [128, 1], f32, kind="ExternalInput")
    thrl_d = nc.dram_tensor("THRL", [128, 1], f32, kind="ExternalInput")
    neg01_d = nc.dram_tensor("NEG01", [128, 1], f32, kind="ExternalInput")
    usr_d = nc.dram_tensor("USR", [128, 128], f32r, kind="ExternalInput")
    onr_d = nc.dram_tensor("ONESR", [128, 128], f32r, kind="ExternalInput")
    stc_d = nc.dram_tensor("STC", [128, 4, 8], f32r, kind="ExternalInput")
    bgw_d = nc.dram_tensor("BGW", [1, 8], f32r, kind="ExternalInput")
    z8_d = nc.dram_tensor("Z8", [128, 8], f32r, kind="ExternalInput")
    out_d = nc.dram_tensor("out", [4, PIX], f32, kind="ExternalOutput")

    Ln = mybir.ActivationFunctionType.Ln
    Exp = mybir.ActivationFunctionType.Exp
    Sign = mybir.ActivationFunctionType.Sign
    MUL = mybir.AluOpType.mult
    LE = mybir.AluOpType.is_le
    GT = mybir.AluOpType.is_gt

    with tile.TileContext(nc) as tc:
        with (
            tc.tile_pool(name="cst", bufs=1) as cst,
            tc.tile_pool(name="io", bufs=4) as io,
            tc.tile_pool(name="wk", bufs=3) as wk,
            tc.tile_pool(name="psq", bufs=6, space="PSUM") as psq,
            tc.tile_pool(name="psr", bufs=2, space="PSUM") as psr,
        ):
            # ---- constants / per-core setup ----
            u2b = cst.tile([128, 128], bf16)
            nc.sync.dma_start(u2b[:], u2b_d[:])
            onb = cst.tile([128, 128], bf16)
            nc.sync.dma_start(onb[:], onb_d[:])
            usr = cst.tile([128, 128], f32r)
            nc.sync.dma_start(usr[:], usr_d[:])
            onr = cst.tile([128, 128], f32r)
            nc.sync.dma_start(onr[:], onr_d[:])
            stc = cst.tile([128, 4, 8], f32r)
            nc.sync.dma_start(stc[:], stc_d[:])
            bgw = cst.tile([1, 8], f32r)
            nc.sync.dma_start(bgw[:], bgw_d[:])
            thrh = cst.tile([128, 1], f32)
            nc.sync.dma_start(thrh[:], thrh_d[:])
            thrl = cst.tile([128, 1], f32)
            nc.sync.dma_start(thrl[:], thrl_d[:])
            neg01 = cst.tile([128, 1], f32)
            nc.sync.dma_start(neg01[:], neg01_d[:])

            par0 = cst.tile([128, 8], f32)
            nc.sync.dma_start(par0[:], p_d[0:128, :])
            par1 = cst.tile([128, 8], f32)
            nc.sync.dma_start(par1[:], p_d[128:256, :])
            st3l = cst.tile([128, 8], f32r)
            nc.sync.dma_start(st3l[:], z8_d[:])
            st3h = cst.tile([128, 8], f32r)
            nc.sync.dma_start(st3h[:], z8_d[:])
            nc.vector.tensor_tensor(st3l[:, 4:5], par0[:, 2:3], par0[:, 3:4], MUL)
            nc.vector.tensor_tensor(st3h[:, 4:5], par1[:, 2:3], par1[:, 3:4], MUL)

            # ---- main loop over pixel tiles ----
            for t in range(NT):
                px = bass.ts(t, F)

                At = io.tile([128, 2, F], f32, tag="A")
                nc.sync.dma_start(
                    At[:], bass.AP(a_d, t * F, [[PIX, 128], [128 * PIX, 2], [1, F]])
                )
                A0, A1 = At[:, 0, :], At[:, 1, :]
                Ct = io.tile([128, 2, 3, F], f32, tag="C")
                for h in range(2):
                    nc.sync.dma_start(
                        Ct[:, h],
                        bass.AP(
                            c_d, h * 128 * 3 * PIX + t * F,
                            [[3 * PIX, 128], [PIX, 3], [1, F]],
                        ),
                    )
                C0, C1 = Ct[:, 0], Ct[:, 1]

                # msign = sign(A - 0.1) in {-1, +1}; the (sign+1)/2 mask
                # rewrite is folded into halved stationaries + per-row
                # thresholds (exact in f32).
                m0 = wk.tile([128, F], bf16, tag="m0")
                nc.scalar.activation(m0[:], A0, Sign, bias=neg01[:])
                m1 = wk.tile([128, F], bf16, tag="m1")
                nc.scalar.activation(m1[:], A1, Sign, bias=neg01[:])

                # E - BIG*m over the stroke axis (exact, bf16 matmul)
                q1 = psq.tile([128, F], f32, tag="big")
                nc.tensor.matmul(q1[:], u2b[:], m1[:], start=True, stop=True)
                ae1 = wk.tile([128, F], f32, tag="ae1")
                nc.vector.scalar_tensor_tensor(ae1[:], q1[:], thrh[:], A1, LE, MUL)
                q0 = psq.tile([128, F], f32, tag="big")
                nc.tensor.matmul(q0[:], u2b[:], m0[:], start=True, stop=False)
                nc.tensor.matmul(q0[:], onb[:], m1[:], start=False, stop=True)
                ae0 = wk.tile([128, F], f32, tag="ae0")
                nc.vector.scalar_tensor_tensor(ae0[:], q0[:], thrl[:], A0, LE, MUL)

                # lg = ln(1 - ae)
                lg0 = wk.tile([128, F], f32r, tag="lg0")
                nc.scalar.activation(lg0[:], ae0[:], Ln, bias=1.0, scale=-1.0)
                lg1 = wk.tile([128, F], f32r, tag="lg1")
                nc.scalar.activation(lg1[:], ae1[:], Ln, bias=1.0, scale=-1.0)

                # suffix log-sums
                Lx1 = psq.tile([128, F], f32, tag="big")
                nc.tensor.matmul(Lx1[:], usr[:], lg1[:], start=True, stop=True)
                Tx1 = wk.tile([128, F], f32, tag="Tx1")
                nc.scalar.activation(Tx1[:], Lx1[:], Exp)
                w1 = wk.tile([128, F], f32r, tag="w1")
                nc.vector.tensor_tensor(w1[:], ae1[:], Tx1[:], MUL)
                Lx0 = psq.tile([128, F], f32, tag="big")
                nc.tensor.matmul(Lx0[:], usr[:], lg0[:], start=True, stop=False)
                nc.tensor.matmul(Lx0[:], onr[:], lg1[:], start=False, stop=True)
                Tx0 = wk.tile([128, F], f32, tag="Tx0")
                nc.scalar.activation(Tx0[:], Lx0[:], Exp)
                w0 = wk.tile([128, F], f32r, tag="w0")
                nc.vector.tensor_tensor(w0[:], ae0[:], Tx0[:], MUL)

                # Z_c = w * color_c  (b channel on gpsimd)
                Z0 = wk.tile([128, 3, F], f32r, tag="Z0")
                nc.vector.tensor_tensor(Z0[:, 0, :], w0[:], C0[:, 0, :], MUL)
                nc.vector.tensor_tensor(Z0[:, 1, :], w0[:], C0[:, 1, :], MUL)
                nc.vector.tensor_tensor(Z0[:, 2, :], w0[:], C0[:, 2, :], MUL)
                Z1 = wk.tile([128, 3, F], f32r, tag="Z1")
                nc.vector.tensor_tensor(Z1[:, 0, :], w1[:], C1[:, 0, :], MUL)
                nc.vector.tensor_tensor(Z1[:, 1, :], w1[:], C1[:, 1, :], MUL)
                nc.vector.tensor_tensor(Z1[:, 2, :], w1[:], C1[:, 2, :], MUL)

                # reductions over the stroke axis into rows:
                # rows 0-2 canvas rgb, row 3 den, row 4 sum(lg)
                red = psr.tile([8, F], f32, tag="red")
                nc.tensor.matmul(red[:], stc[:, 0, :], Z1[:, 0, :], start=True, stop=False)
                nc.tensor.matmul(red[:], stc[:, 0, :], Z0[:, 0, :], start=False, stop=False)
                nc.tensor.matmul(red[:], stc[:, 1, :], Z1[:, 1, :], start=False, stop=False)
                nc.tensor.matmul(red[:], stc[:, 1, :], Z0[:, 1, :], start=False, stop=False)
                nc.tensor.matmul(red[:], stc[:, 2, :], Z1[:, 2, :], start=False, stop=False)
                nc.tensor.matmul(red[:], stc[:, 2, :], Z0[:, 2, :], start=False, stop=False)
                nc.tensor.matmul(red[:], st3h[:], w1[:], start=False, stop=False)
                nc.tensor.matmul(red[:], st3l[:], w0[:], start=False, stop=False)
                nc.tensor.matmul(red[:], stc[:, 3, :], lg1[:], start=False, stop=False)
                nc.tensor.matmul(red[:], stc[:, 3, :], lg0[:], start=False, stop=False)

                # background transmittance, added to rows 0-3 via rank-1 matmul
                bg = wk.tile([1, F], f32r, tag="bg")
                nc.scalar.activation(bg[:], red[0:1, :], Exp)
                nc.tensor.matmul(red[:], bgw[:], bg[:], start=False, stop=True)

                outt = wk.tile([8, F], f32, tag="outt")
                nc.scalar.copy(outt[:], red[:])
                nc.sync.dma_start(out_d[:, px], outt[1:5, :])

    nc.compile()
    return nc


def _get_program():
    if "nc" not in _CACHED:
        _CACHED["nc"] = _build_program()
    return _CACHED["nc"]


def _consts():
    if "consts" in _CACHED:
        return _CACHED["consts"]
    tri = np.tril(np.ones((128, 128), np.float32), -1)
    # halved: the matmul consumes msign in {-1,+1}; q = U2h@msign + r
    u2 = 0.5 * tri - (BIG / 2) * np.eye(128, dtype=np.float32)
    ones = np.ones((128, 128), np.float32)
    r = u2.sum(axis=0).astype(np.float32).reshape(128, 1)   # per-row offset
    thrh = np.float32(THRESH) - r
    thrl = np.float32(THRESH) - r - np.float32(64.0)
    stc = np.zeros((128, 4, 8), np.float32)
    stc[:, 0, 1] = 1.0   # canvas r -> row 1
    stc[:, 1, 2] = 1.0   # canvas g -> row 2
    stc[:, 2, 3] = 1.0   # canvas b -> row 3
    stc[:, 3, 0] = 1.0   # sum(lg) -> row 0
    bgw = np.zeros((1, 8), np.float32)
    bgw[0, 1:5] = 1.0
    consts = {
        "U2B": u2.astype(ml_dtypes.bfloat16),
        "ONESB": (0.5 * ones).astype(ml_dtypes.bfloat16),
        "THRH": thrh,
        "THRL": thrl,
        "NEG01": np.full((128, 1), -np.float32(0.1), np.float32),
        "USR": tri,
        "ONESR": ones,
        "STC": stc,
        "BGW": bgw,
        "Z8": np.zeros((128, 8), np.float32),
    }
    _CACHED["consts"] = consts
    return consts


def _make_in_maps(color_stroke, alpha, params):
    consts = _consts()
    in_maps = []
    for core in range(8):
        b, half = core // 2, core % 2
        r0 = half * (H // 2)
        a = np.ascontiguousarray(
            alpha[b, :, 0, r0 : r0 + H // 2, :].reshape(N, PIX)
        )
        c = np.ascontiguousarray(
            color_stroke[b, :, :, r0 : r0 + H // 2, :].reshape(N, 3, PIX)
        )
        p = np.ascontiguousarray(params[b])
        in_maps.append({"alpha": a, "color": c, "par": p, **consts})
    return in_maps


def kernel(color_stroke, alpha, params, _trace=False, _trace_kwargs=None):
    color_stroke = np.asarray(color_stroke, dtype=np.float32)
    alpha = np.asarray(alpha, dtype=np.float32)
    params = np.asarray(params, dtype=np.float32)

    nc = _get_program()
    in_maps = _make_in_maps(color_stroke, alpha, params)
    res = run_bass_kernel_spmd(
        nc, in_maps, list(range(8)), trace=_trace, **(_trace_kwargs or {})
    )
    _CACHED["last_result"] = res

    canvas = np.empty((B, 3, H, W), np.float32)
    den = np.empty((B, 1, H, W), np.float32)
    for core in range(8):
        b, half = core // 2, core % 2
        r0 = half * (H // 2)
        o = res.results[core]["out"]
        canvas[b, :, r0 : r0 + H // 2, :] = o[0:3].reshape(3, H // 2, W)
        den[b, 0, r0 : r0 + H // 2, :] = o[3].reshape(H // 2, W)
    return canvas, den


# revision 11
# speedup vs baseline: 1.0176x; 1.0176x over previous
"""Trainium2 Bass kernel for AttnPainterOilDensity (per-pixel top-10 stroke
selection + back-to-front alpha compositing).

Math (per pixel, strokes n = 0..255):
  m_n   = alpha_n > 0.1
  E_n   = #{k > n : m_k}                      (visible strokes in front)
  sel_n = m_n and E_n <= 9                    (the last 10 visible strokes)
  ae_n  = alpha_n * sel_n
  lg_n  = ln(1 - ae_n)                        (0 for unselected)
  Lx_n  = sum_{k>n} lg_k ;  Texcl_n = exp(Lx_n)
  w_n   = ae_n * Texcl_n                      (compositing weight)
  canvas_c = sum_n w_n * color_{n,c} + exp(sum_n lg_n)
  den      = sum_n w_n * s_n        + exp(sum_n lg_n),  s_n = p2_n * p3_n

Layout: stroke-major tiles [128 strokes, 512 pixels]; the per-pixel suffix
counts/sums over the stroke (partition) axis run on the PE via triangular
constant stationaries; selection via one fused scalar_tensor_tensor; ln/exp
on ACT; per-stroke reductions back to per-pixel rows via small matmuls.

Sharding: 8 cores = (batch b = core//2) x (half of the 128x128 plane).
"""

import numpy as np
import ml_dtypes

import concourse.bacc as bacc
import concourse.bass as bass
import concourse.tile as tile
from concourse import mybir
from concourse.bass_utils import run_bass_kernel_spmd


def _patch_act_tables():
    # Force Ln and Exp onto the shared natural_log_exp_and_others set so the
    # per-tile Ln -> Exp alternation doesn't reload ACT tables 2x per tile.
    if _CACHED.get("act_patched"):
        return
    import concourse.hw_specs as hw_specs
    orig = hw_specs.get_activation_tables

    def patched(arch):
        tables = dict(orig(arch))
        ln = mybir.ActivationFunctionType.Ln
        ex = mybir.ActivationFunctionType.Exp
        for name, fns in tables.items():
            if name != "natural_log_exp_and_others":
                tables[name] = fns - {ln, ex}
        return tables

    hw_specs.get_activation_tables = patched
    bacc.get_activation_tables = patched
    _CACHED["act_patched"] = True

B, N, H, W = 4, 256, 128, 128
PIX = H * W // 2          # pixels per core (half plane) = 8192
F = 512                   # pixels per tile
NT = PIX // F             # 16 tiles
BIG = 1024.0
THRESH = -1014.5          # q <= 9 - BIG

f32 = mybir.dt.float32
f32r = mybir.dt.float32r
bf16 = mybir.dt.bfloat16

_CACHED = {}


def _build_program():
    _patch_act_tables()
    nc = bacc.Bacc("TRN2", target_bir_lowering=False, debug=False, num_devices=8)

    a_d = nc.dram_tensor("alpha", [N, PIX], f32, kind="ExternalInput")
    c_d = nc.dram_tensor("color", [N, 3, PIX], f32, kind="ExternalInput")
    p_d = nc.dram_tensor("par", [N, 8], f32, kind="ExternalInput")
    u2b_d = nc.dram_tensor("U2B", [128, 128], bf16, kind="ExternalInput")
    onb_d = nc.dram_tensor("ONESB", [128, 128], bf16, kind="ExternalInput")
    thrh_d = nc.dram_tensor("THRH", [128, 1], f32, kind="ExternalInput")
    thrl_d = nc.dram_tensor("THRL", [128, 1], f32, kind="ExternalInput")
    neg01_d = nc.dram_tensor("NEG01", [128, 1], f32, kind="ExternalInput")
    usr_d = nc.dram_tensor("USR", [128, 128], f32r, kind="ExternalInput")
    onr_d = nc.dram_tensor("ONESR", [128, 128], f32r, kind="ExternalInput")
    stc_d = nc.dram_tensor("STC", [128, 4, 8], f32r, kind="ExternalInput")
    bgw_d = nc.dram_tensor("BGW", [1, 8], f32r, kind="ExternalInput")
    z8_d = nc.dram_tensor("Z8", [128, 8], f32r, kind="ExternalInput")
    out_d = nc.dram_tensor("out", [4, PIX], f32, kind="ExternalOutput")

    Ln = mybir.ActivationFunctionType.Ln
    Exp = mybir.ActivationFunctionType.Exp
    Sign = mybir.ActivationFunctionType.Sign
    MUL = mybir.AluOpType.mult
    LE = mybir.AluOpType.is_le
    GT = mybir.AluOpType.is_gt

    with tile.TileContext(nc) as tc:
        with (
            tc.tile_pool(name="cst", bufs=1) as cst,
            tc.tile_pool(name="io", bufs=5) as io,
            tc.tile_pool(name="wk", bufs=3) as wk,
            tc.tile_pool(name="psq", bufs=6, space="PSUM") as psq,
            tc.tile_pool(name="psr", bufs=2, space="PSUM") as psr,
        ):
            # ---- constants / per-core setup ----
            u2b = cst.tile([128, 128], bf16)
            nc.sync.dma_start(u2b[:], u2b_d[:])
            onb = cst.tile([128, 128], bf16)
            nc.sync.dma_start(onb[:], onb_d[:])
            usr = cst.tile([128, 128], f32r)
            nc.sync.dma_start(usr[:], usr_d[:])
            onr = cst.tile([128, 128], f32r)
            nc.sync.dma_start(onr[:], onr_d[:])
            stc = cst.tile([128, 4, 8], f32r)
            nc.sync.dma_start(stc[:], stc_d[:])
            bgw = cst.tile([1, 8], f32r)
            nc.sync.dma_start(bgw[:], bgw_d[:])
            thrh = cst.tile([128, 1], f32)
            nc.sync.dma_start(thrh[:], thrh_d[:])
            thrl = cst.tile([128, 1], f32)
            nc.sync.dma_start(thrl[:], thrl_d[:])
            neg01 = cst.tile([128, 1], f32)
            nc.sync.dma_start(neg01[:], neg01_d[:])

            par0 = cst.tile([128, 8], f32)
            nc.sync.dma_start(par0[:], p_d[0:128, :])
            par1 = cst.tile([128, 8], f32)
            nc.sync.dma_start(par1[:], p_d[128:256, :])
            st3l = cst.tile([128, 8], f32r)
            nc.sync.dma_start(st3l[:], z8_d[:])
            st3h = cst.tile([128, 8], f32r)
            nc.sync.dma_start(st3h[:], z8_d[:])
            nc.vector.tensor_tensor(st3l[:, 4:5], par0[:, 2:3], par0[:, 3:4], MUL)
            nc.vector.tensor_tensor(st3h[:, 4:5], par1[:, 2:3], par1[:, 3:4], MUL)

            # ---- main loop over pixel tiles ----
            for t in range(NT):
                px = bass.ts(t, F)

                At = io.tile([128, 2, F], f32, tag="A")
                Ct = io.tile([128, 2, 3, F], f32, tag="C")
                with tc.high_priority(offset=60):
                    nc.sync.dma_start(
                        At[:],
                        bass.AP(a_d, t * F, [[PIX, 128], [128 * PIX, 2], [1, F]]),
                    )
                    for h in range(2):
                        nc.sync.dma_start(
                            Ct[:, h],
                            bass.AP(
                                c_d, h * 128 * 3 * PIX + t * F,
                                [[3 * PIX, 128], [PIX, 3], [1, F]],
                            ),
                        )
                A0, A1 = At[:, 0, :], At[:, 1, :]
                C0, C1 = Ct[:, 0], Ct[:, 1]

                # msign = sign(A - 0.1) in {-1, +1}; the (sign+1)/2 mask
                # rewrite is folded into halved stationaries + per-row
                # thresholds (exact in f32).
                m0 = wk.tile([128, F], bf16, tag="m0")
                nc.scalar.activation(m0[:], A0, Sign, bias=neg01[:])
                m1 = wk.tile([128, F], bf16, tag="m1")
                nc.scalar.activation(m1[:], A1, Sign, bias=neg01[:])

                # E - BIG*m over the stroke axis (exact, bf16 matmul)
                q1 = psq.tile([128, F], f32, tag="big")
                nc.tensor.matmul(q1[:], u2b[:], m1[:], start=True, stop=True)
                ae1 = wk.tile([128, F], f32, tag="ae1")
                nc.vector.scalar_tensor_tensor(ae1[:], q1[:], thrh[:], A1, LE, MUL)
                q0 = psq.tile([128, F], f32, tag="big")
                nc.tensor.matmul(q0[:], u2b[:], m0[:], start=True, stop=False)
                nc.tensor.matmul(q0[:], onb[:], m1[:], start=False, stop=True)
                ae0 = wk.tile([128, F], f32, tag="ae0")
                nc.vector.scalar_tensor_tensor(ae0[:], q0[:], thrl[:], A0, LE, MUL)

                # lg = ln(1 - ae)
                lg0 = wk.tile([128, F], f32r, tag="lg0")
                nc.scalar.activation(lg0[:], ae0[:], Ln, bias=1.0, scale=-1.0)
                lg1 = wk.tile([128, F], f32r, tag="lg1")
                nc.scalar.activation(lg1[:], ae1[:], Ln, bias=1.0, scale=-1.0)

                # suffix log-sums
                Lx1 = psq.tile([128, F], f32, tag="big")
                nc.tensor.matmul(Lx1[:], usr[:], lg1[:], start=True, stop=True)
                Tx1 = wk.tile([128, F], f32, tag="Tx1")
                nc.scalar.activation(Tx1[:], Lx1[:], Exp)
                w1 = wk.tile([128, F], f32r, tag="w1")
                nc.vector.tensor_tensor(w1[:], ae1[:], Tx1[:], MUL)
                Lx0 = psq.tile([128, F], f32, tag="big")
                nc.tensor.matmul(Lx0[:], usr[:], lg0[:], start=True, stop=False)
                nc.tensor.matmul(Lx0[:], onr[:], lg1[:], start=False, stop=True)
                Tx0 = wk.tile([128, F], f32, tag="Tx0")
                nc.scalar.activation(Tx0[:], Lx0[:], Exp)
                w0 = wk.tile([128, F], f32r, tag="w0")
                nc.vector.tensor_tensor(w0[:], ae0[:], Tx0[:], MUL)

                # Z_c = w * color_c  (b channel on gpsimd)
                Z0 = wk.tile([128, 3, F], f32r, tag="Z0")
                nc.vector.tensor_tensor(Z0[:, 0, :], w0[:], C0[:, 0, :], MUL)
                nc.vector.tensor_tensor(Z0[:, 1, :], w0[:], C0[:, 1, :], MUL)
                nc.vector.tensor_tensor(Z0[:, 2, :], w0[:], C0[:, 2, :], MUL)
                Z1 = wk.tile([128, 3, F], f32r, tag="Z1")
                nc.vector.tensor_tensor(Z1[:, 0, :], w1[:], C1[:, 0, :], MUL)
                nc.vector.tensor_tensor(Z1[:, 1, :], w1[:], C1[:, 1, :], MUL)
                nc.vector.tensor_tensor(Z1[:, 2, :], w1[:], C1[:, 2, :], MUL)

                # reductions over the stroke axis into rows:
                # rows 0-2 canvas rgb, row 3 den, row 4 sum(lg)
                red = psr.tile([8, F], f32, tag="red")
                nc.tensor.matmul(red[:], stc[:, 0, :], Z1[:, 0, :], start=True, stop=False)
                nc.tensor.matmul(red[:], stc[:, 0, :], Z0[:, 0, :], start=False, stop=False)
                nc.tensor.matmul(red[:], stc[:, 1, :], Z1[:, 1, :], start=False, stop=False)
                nc.tensor.matmul(red[:], stc[:, 1, :], Z0[:, 1, :], start=False, stop=False)
                nc.tensor.matmul(red[:], stc[:, 2, :], Z1[:, 2, :], start=False, stop=False)
                nc.tensor.matmul(red[:], stc[:, 2, :], Z0[:, 2, :], start=False, stop=False)
                nc.tensor.matmul(red[:], st3h[:], w1[:], start=False, stop=False)
                nc.tensor.matmul(red[:], st3l[:], w0[:], start=False, stop=False)
                nc.tensor.matmul(red[:], stc[:, 3, :], lg1[:], start=False, stop=False)
                nc.tensor.matmul(red[:], stc[:, 3, :], lg0[:], start=False, stop=False)

                # background transmittance, added to rows 0-3 via rank-1 matmul
                bg = wk.tile([1, F], f32r, tag="bg")
                nc.scalar.activation(bg[:], red[0:1, :], Exp)
                nc.tensor.matmul(red[:], bgw[:], bg[:], start=False, stop=True)

                outt = wk.tile([8, F], f32, tag="outt")
                nc.scalar.copy(outt[:], red[:])
                nc.sync.dma_start(out_d[:, px], outt[1:5, :])

    nc.compile()
    return nc


def _get_program():
    if "nc" not in _CACHED:
        _CACHED["nc"] = _build_program()
    return _CACHED["nc"]


def _consts():
    if "consts" in _CACHED:
        return _CACHED["consts"]
    tri = np.tril(np.ones((128, 128), np.float32), -1)
    # halved: the matmul consumes msign in {-1,+1}; q = U2h@msign + r
    u2 = 0.5 * tri - (BIG / 2) * np.eye(128, dtype=np.float32)
    ones = np.ones((128, 128), np.float32)
    r = u2.sum(axis=0).astype(np.float32).reshape(128, 1)   # per-row offset
    thrh = np.float32(THRESH) - r
    thrl = np.float32(THRESH) - r - np.float32(64.0)
    stc = np.zeros((128, 4, 8), np.float32)
    stc[:, 0, 1] = 1.0   # canvas r -> row 1
    stc[:, 1, 2] = 1.0   # canvas g -> row 2
    stc[:, 2, 3] = 1.0   # canvas b -> row 3
    stc[:, 3, 0] = 1.0   # sum(lg) -> row 0
    bgw = np.zeros((1, 8), np.float32)
    bgw[0, 1:5] = 1.0
    consts = {
        "U2B": u2.astype(ml_dtypes.bfloat16),
        "ONESB": (0.5 * ones).astype(ml_dtypes.bfloat16),
        "THRH": thrh,
        "THRL": thrl,
        "NEG01": np.full((128, 1), -np.float32(0.1), np.float32),
        "USR": tri,
        "ONESR": ones,
        "STC": stc,
        "BGW": bgw,
        "Z8": np.zeros((128, 8), np.float32),
    }
    _CACHED["consts"] = consts
    return consts


def _make_in_maps(color_stroke, alpha, params):
    consts = _consts()
    in_maps = []
    for core in range(8):
        b, half = core // 2, core % 2
        r0 = half * (H // 2)
        a = np.ascontiguousarray(
            alpha[b, :, 0, r0 : r0 + H // 2, :].reshape(N, PIX)
        )
        c = np.ascontiguousarray(
            color_stroke[b, :, :, r0 : r0 + H // 2, :].reshape(N, 3, PIX)
        )
        p = np.ascontiguousarray(params[b])
        in_maps.append({"alpha": a, "color": c, "par": p, **consts})
    return in_maps


def kernel(color_stroke, alpha, params, _trace=False, _trace_kwargs=None):
    color_stroke = np.asarray(color_stroke, dtype=np.float32)
    alpha = np.asarray(alpha, dtype=np.float32)
    params = np.asarray(params, dtype=np.float32)

    nc = _get_program()
    in_maps = _make_in_maps(color_stroke, alpha, params)
    res = run_bass_kernel_spmd(
        nc, in_maps, list(range(8)), trace=_trace, **(_trace_kwargs or {})
    )
    _CACHED["last_result"] = res

    canvas = np.empty((B, 3, H, W), np.float32)
    den = np.empty((B, 1, H, W), np.float32)
    for core in range(8):
        b, half = core // 2, core % 2
        r0 = half * (H // 2)
        o = res.results[core]["out"]
        canvas[b, :, r0 : r0 + H // 2, :] = o[0:3].reshape(3, H // 2, W)
        den[b, 0, r0 : r0 + H // 2, :] = o[3].reshape(H // 2, W)
    return canvas, den


# revision 12
# speedup vs baseline: 1.0578x; 1.0395x over previous
"""Trainium2 Bass kernel for AttnPainterOilDensity (per-pixel top-10 stroke
selection + back-to-front alpha compositing).

Math (per pixel, strokes n = 0..255):
  m_n   = alpha_n > 0.1
  E_n   = #{k > n : m_k}                      (visible strokes in front)
  sel_n = m_n and E_n <= 9                    (the last 10 visible strokes)
  ae_n  = alpha_n * sel_n
  lg_n  = ln(1 - ae_n)                        (0 for unselected)
  Lx_n  = sum_{k>n} lg_k ;  Texcl_n = exp(Lx_n)
  w_n   = ae_n * Texcl_n                      (compositing weight)
  canvas_c = sum_n w_n * color_{n,c} + exp(sum_n lg_n)
  den      = sum_n w_n * s_n        + exp(sum_n lg_n),  s_n = p2_n * p3_n

Layout: stroke-major tiles [128 strokes, 512 pixels]; the per-pixel suffix
counts/sums over the stroke (partition) axis run on the PE via triangular
constant stationaries; selection via one fused scalar_tensor_tensor; ln/exp
on ACT; per-stroke reductions back to per-pixel rows via small matmuls.

Sharding: 8 cores = (batch b = core//2) x (half of the 128x128 plane).
"""

import numpy as np
import ml_dtypes

import concourse.bacc as bacc
import concourse.bass as bass
import concourse.tile as tile
from concourse import mybir
from concourse.bass_utils import run_bass_kernel_spmd


def _patch_act_tables():
    # Force Ln and Exp onto the shared natural_log_exp_and_others set so the
    # per-tile Ln -> Exp alternation doesn't reload ACT tables 2x per tile.
    if _CACHED.get("act_patched"):
        return
    import concourse.hw_specs as hw_specs
    orig = hw_specs.get_activation_tables

    def patched(arch):
        tables = dict(orig(arch))
        ln = mybir.ActivationFunctionType.Ln
        ex = mybir.ActivationFunctionType.Exp
        for name, fns in tables.items():
            if name != "natural_log_exp_and_others":
                tables[name] = fns - {ln, ex}
        return tables

    hw_specs.get_activation_tables = patched
    bacc.get_activation_tables = patched
    _CACHED["act_patched"] = True

B, N, H, W = 4, 256, 128, 128
PIX = H * W // 2          # pixels per core (half plane) = 8192
F = 512                   # pixels per tile
NT = PIX // F             # 16 tiles
BIG = 1024.0
THRESH = -1014.5          # q <= 9 - BIG

f32 = mybir.dt.float32
f32r = mybir.dt.float32r
bf16 = mybir.dt.bfloat16

_CACHED = {}


def _build_program():
    _patch_act_tables()
    nc = bacc.Bacc("TRN2", target_bir_lowering=False, debug=False, num_devices=8)

    a_d = nc.dram_tensor("alpha", [N, PIX], f32, kind="ExternalInput")
    c_d = nc.dram_tensor("color", [N, 3, PIX], f32, kind="ExternalInput")
    p_d = nc.dram_tensor("par", [N, 8], f32, kind="ExternalInput")
    u2b_d = nc.dram_tensor("U2B", [128, 128], bf16, kind="ExternalInput")
    onb_d = nc.dram_tensor("ONESB", [128, 128], bf16, kind="ExternalInput")
    thr_d = nc.dram_tensor("THR", [128, 1], f32, kind="ExternalInput")
    p64_d = nc.dram_tensor("P64", [1, 128], bf16, kind="ExternalInput")
    neg01_d = nc.dram_tensor("NEG01", [128, 1], f32, kind="ExternalInput")
    usr_d = nc.dram_tensor("USR", [128, 128], f32r, kind="ExternalInput")
    onr_d = nc.dram_tensor("ONESR", [128, 128], f32r, kind="ExternalInput")
    stc_d = nc.dram_tensor("STC", [128, 4, 8], f32r, kind="ExternalInput")
    bgw_d = nc.dram_tensor("BGW", [1, 8], f32r, kind="ExternalInput")
    z8_d = nc.dram_tensor("Z8", [128, 8], f32r, kind="ExternalInput")
    out_d = nc.dram_tensor("out", [4, PIX], f32, kind="ExternalOutput")

    Ln = mybir.ActivationFunctionType.Ln
    Exp = mybir.ActivationFunctionType.Exp
    Sign = mybir.ActivationFunctionType.Sign
    MUL = mybir.AluOpType.mult
    LE = mybir.AluOpType.is_le
    GT = mybir.AluOpType.is_gt

    with tile.TileContext(nc) as tc:
        with (
            tc.tile_pool(name="cst", bufs=1) as cst,
            tc.tile_pool(name="io", bufs=5) as io,
            tc.tile_pool(name="wk", bufs=3) as wk,
            tc.tile_pool(name="psq", bufs=2, space="PSUM") as psq,
            tc.tile_pool(name="psl", bufs=1, space="PSUM") as psl,
            tc.tile_pool(name="psr", bufs=2, space="PSUM") as psr,
        ):
            # ---- constants / per-core setup ----
            u2b = cst.tile([128, 128], bf16)
            nc.sync.dma_start(u2b[:], u2b_d[:])
            onb = cst.tile([128, 128], bf16)
            nc.sync.dma_start(onb[:], onb_d[:])
            usr = cst.tile([128, 128], f32r)
            nc.sync.dma_start(usr[:], usr_d[:])
            onr = cst.tile([128, 128], f32r)
            nc.sync.dma_start(onr[:], onr_d[:])
            stc = cst.tile([128, 4, 8], f32r)
            nc.sync.dma_start(stc[:], stc_d[:])
            bgw = cst.tile([1, 8], f32r)
            nc.sync.dma_start(bgw[:], bgw_d[:])
            thr = cst.tile([128, 1], f32)
            nc.sync.dma_start(thr[:], thr_d[:])
            p64 = cst.tile([1, 128], bf16)
            nc.sync.dma_start(p64[:], p64_d[:])
            onerow = cst.tile([1, F], bf16)
            nc.vector.memset(onerow[:], 1.0)
            neg01 = cst.tile([128, 1], f32)
            nc.sync.dma_start(neg01[:], neg01_d[:])

            par0 = cst.tile([128, 8], f32)
            nc.sync.dma_start(par0[:], p_d[0:128, :])
            par1 = cst.tile([128, 8], f32)
            nc.sync.dma_start(par1[:], p_d[128:256, :])
            st3l = cst.tile([128, 8], f32r)
            nc.sync.dma_start(st3l[:], z8_d[:])
            st3h = cst.tile([128, 8], f32r)
            nc.sync.dma_start(st3h[:], z8_d[:])
            nc.vector.tensor_tensor(st3l[:, 4:5], par0[:, 2:3], par0[:, 3:4], MUL)
            nc.vector.tensor_tensor(st3h[:, 4:5], par1[:, 2:3], par1[:, 3:4], MUL)

            # ---- main loop over pixel tiles ----
            for t in range(NT):
                px = bass.ts(t, F)

                At = io.tile([128, 2, F], f32, tag="A")
                Ct = io.tile([128, 2, 3, F], f32, tag="C")
                with tc.high_priority(offset=60):
                    nc.sync.dma_start(
                        At[:],
                        bass.AP(a_d, t * F, [[PIX, 128], [128 * PIX, 2], [1, F]]),
                    )
                    for h in range(2):
                        nc.sync.dma_start(
                            Ct[:, h],
                            bass.AP(
                                c_d, h * 128 * 3 * PIX + t * F,
                                [[3 * PIX, 128], [PIX, 3], [1, F]],
                            ),
                        )

                # msign = sign(A - 0.1) in {-1,+1}; mask algebra folded into
                # halved stationaries, per-row thresholds and a +64 rank-1.
                ms = wk.tile([128, 2, F], bf16, tag="ms")
                nc.scalar.activation(ms[:], At[:], Sign, bias=neg01[:])

                q2 = psq.tile([128, 2, F], f32, tag="q2")
                nc.tensor.matmul(q2[:, 1, :], u2b[:], ms[:, 1, :], start=True, stop=True)
                nc.tensor.matmul(q2[:, 0, :], u2b[:], ms[:, 0, :], start=True, stop=False)
                nc.tensor.matmul(q2[:, 0, :], onb[:], ms[:, 1, :], start=False, stop=False)
                nc.tensor.matmul(q2[:, 0, :], p64[:], onerow[:], start=False, stop=True)

                # ae = (q <= thr) * A over both halves at once
                aet = wk.tile([128, 2, F], f32, tag="aet")
                nc.vector.scalar_tensor_tensor(aet[:], q2[:], thr[:], At[:], LE, MUL)

                # lg = ln(1 - ae)
                lgt = wk.tile([128, 2, F], f32r, tag="lgt")
                nc.scalar.activation(lgt[:], aet[:], Ln, bias=1.0, scale=-1.0)

                # suffix log-sums
                Lx = psl.tile([128, 2, F], f32, tag="Lx")
                nc.tensor.matmul(Lx[:, 1, :], usr[:], lgt[:, 1, :], start=True, stop=True)
                nc.tensor.matmul(Lx[:, 0, :], usr[:], lgt[:, 0, :], start=True, stop=False)
                nc.tensor.matmul(Lx[:, 0, :], onr[:], lgt[:, 1, :], start=False, stop=True)

                Txt = wk.tile([128, 2, F], f32, tag="Txt")
                nc.scalar.activation(Txt[:], Lx[:], Exp)

                # w = ae * Texcl
                wt = wk.tile([128, 2, 1, F], f32r, tag="wt")
                nc.vector.tensor_tensor(wt[:, :, 0, :], aet[:], Txt[:], MUL)

                # Z = w (broadcast over channel) * color, one op
                Zt = wk.tile([128, 2, 3, F], f32r, tag="Zt")
                nc.vector.tensor_tensor(
                    Zt[:], wt[:].to_broadcast([128, 2, 3, F]), Ct[:], MUL
                )

                # reductions: row0 sum(lg), rows 1-3 canvas rgb, row 4 den
                red = psr.tile([8, F], f32, tag="red")
                nc.tensor.matmul(red[:], stc[:, 0, :], Zt[:, 1, 0, :], start=True, stop=False)
                nc.tensor.matmul(red[:], stc[:, 0, :], Zt[:, 0, 0, :], start=False, stop=False)
                nc.tensor.matmul(red[:], stc[:, 1, :], Zt[:, 1, 1, :], start=False, stop=False)
                nc.tensor.matmul(red[:], stc[:, 1, :], Zt[:, 0, 1, :], start=False, stop=False)
                nc.tensor.matmul(red[:], stc[:, 2, :], Zt[:, 1, 2, :], start=False, stop=False)
                nc.tensor.matmul(red[:], stc[:, 2, :], Zt[:, 0, 2, :], start=False, stop=False)
                nc.tensor.matmul(red[:], st3h[:], wt[:, 1, 0, :], start=False, stop=False)
                nc.tensor.matmul(red[:], st3l[:], wt[:, 0, 0, :], start=False, stop=False)
                nc.tensor.matmul(red[:], stc[:, 3, :], lgt[:, 1, :], start=False, stop=False)
                nc.tensor.matmul(red[:], stc[:, 3, :], lgt[:, 0, :], start=False, stop=False)

                # background transmittance onto rows 1-4 via rank-1 matmul
                bg = wk.tile([1, F], f32r, tag="bg")
                nc.scalar.activation(bg[:], red[0:1, :], Exp)
                nc.tensor.matmul(red[:], bgw[:], bg[:], start=False, stop=True)

                outt = wk.tile([8, F], f32, tag="outt")
                nc.scalar.copy(outt[:], red[:])
                nc.sync.dma_start(out_d[:, px], outt[1:5, :])

    nc.compile()
    return nc


def _get_program():
    if "nc" not in _CACHED:
        _CACHED["nc"] = _build_program()
    return _CACHED["nc"]


def _consts():
    if "consts" in _CACHED:
        return _CACHED["consts"]
    tri = np.tril(np.ones((128, 128), np.float32), -1)
    # halved: the matmul consumes msign in {-1,+1}; q = U2h@msign + r
    u2 = 0.5 * tri - (BIG / 2) * np.eye(128, dtype=np.float32)
    ones = np.ones((128, 128), np.float32)
    r = u2.sum(axis=0).astype(np.float32).reshape(128, 1)   # per-row offset
    thr = np.float32(THRESH) - r
    stc = np.zeros((128, 4, 8), np.float32)
    stc[:, 0, 1] = 1.0   # canvas r -> row 1
    stc[:, 1, 2] = 1.0   # canvas g -> row 2
    stc[:, 2, 3] = 1.0   # canvas b -> row 3
    stc[:, 3, 0] = 1.0   # sum(lg) -> row 0
    bgw = np.zeros((1, 8), np.float32)
    bgw[0, 1:5] = 1.0
    consts = {
        "U2B": u2.astype(ml_dtypes.bfloat16),
        "ONESB": (0.5 * ones).astype(ml_dtypes.bfloat16),
        "THR": thr,
        "P64": np.full((1, 128), 64.0, ml_dtypes.bfloat16),
        "NEG01": np.full((128, 1), -np.float32(0.1), np.float32),
        "USR": tri,
        "ONESR": ones,
        "STC": stc,
        "BGW": bgw,
        "Z8": np.zeros((128, 8), np.float32),
    }
    _CACHED["consts"] = consts
    return consts


def _make_in_maps(color_stroke, alpha, params):
    consts = _consts()
    in_maps = []
    for core in range(8):
        b, half = core // 2, core % 2
        r0 = half * (H // 2)
        a = np.ascontiguousarray(
            alpha[b, :, 0, r0 : r0 + H // 2, :].reshape(N, PIX)
        )
        c = np.ascontiguousarray(
            color_stroke[b, :, :, r0 : r0 + H // 2, :].reshape(N, 3, PIX)
        )
        p = np.ascontiguousarray(params[b])
        in_maps.append({"alpha": a, "color": c, "par": p, **consts})
    return in_maps


def kernel(color_stroke, alpha, params, _trace=False, _trace_kwargs=None):
    color_stroke = np.asarray(color_stroke, dtype=np.float32)
    alpha = np.asarray(alpha, dtype=np.float32)
    params = np.asarray(params, dtype=np.float32)

    nc = _get_program()
    in_maps = _make_in_maps(color_stroke, alpha, params)
    res = run_bass_kernel_spmd(
        nc, in_maps, list(range(8)), trace=_trace, **(_trace_kwargs or {})
    )
    _CACHED["last_result"] = res

    canvas = np.empty((B, 3, H, W), np.float32)
    den = np.empty((B, 1, H, W), np.float32)
    for core in range(8):
        b, half = core // 2, core % 2
        r0 = half * (H // 2)
        o = res.results[core]["out"]
        canvas[b, :, r0 : r0 + H // 2, :] = o[0:3].reshape(3, H // 2, W)
        den[b, 0, r0 : r0 + H // 2, :] = o[3].reshape(H // 2, W)
    return canvas, den
